# revision 9
# baseline (speedup 1.0000x reference)
"""Trainium2 Bass kernel for DepthwiseSeparableConv3d (inference).

Problem: x[2,48,48,48,64] -> dw3x3x3 depthwise + BN + ReLU -> 1x1x1 conv
(64->128) + BN + ReLU -> z[2,48,48,48,128], all f32.

Strategy (8 NeuronCores, data-parallel over (b,d) slabs, 12 slabs/core):
 - Host pre-pads D (1-slab halo per side, zero at batch edges) and H/W
   (SAME padding) so the device kernel is a pure VALID 3x3x3 conv.
 - Depthwise conv runs on TensorE as a block-Toeplitz matmul:
   K=112 partitions = (8 channels x 14 W-inputs),
   M=96 partitions  = (8 channels x 12 W-outputs).
   The 3 W-taps live in the Toeplitz weight; the 9 (dz,dy) taps are
   PSUM-accumulated matmuls against free-dim-shifted views of the same
   SBUF tile (shifting free dims is free in an access pattern).
 - BN1+ReLU is one ScalarE activation (per-partition scale/bias).
 - A per-channel-group SBUF->SBUF DMA regroups (c,w)-partitions into
   pure-channel partitions (contiguous 576-elem blocks on both sides).
 - Pointwise 64->128 is a plain matmul; BN2+ReLU is one activation.
 - Output stays [f, positions] on device; host transposes to NDHWC.
"""

import os
import sys

for _p in ("/opt/trn_rl_repo", "/opt/pypackages"):
    if _p not in sys.path:
        sys.path.insert(0, _p)

import numpy as np
import ml_dtypes

import concourse.bass as bass
import concourse.tile as tile
from concourse import bacc, mybir
from concourse.bass_utils import run_bass_kernel_spmd

# ----- problem constants (hardcoded per spec) -----
B, D, H, W, C, F = 2, 48, 48, 48, 64, 128
EPS = 1e-3
N_CORES = 8
DPC = (B * D) // N_CORES          # d-slabs per core = 12
CG = 8                            # channels per depthwise group
NG = C // CG                      # 8 groups
WT = 4                            # W tiles
WO = W // WT                      # 12 outputs per W tile
WI = WO + 2                       # 14 inputs per W tile
KP = CG * WI                      # 112 K partitions
MP = CG * WO                      # 96 M partitions
DH = DPC * H                      # 576 (d,h) positions per W value
NHALF = 2                         # split (d,h) into two 288-col matmuls
NCOL = DH // NHALF                # 288
NPOS = DPC * H * W                # 27648 positions per core
ZCHUNK = 4                        # PW chunks per output DMA

BF16 = mybir.dt.bfloat16
F32 = mybir.dt.float32

_COMPILED = None


def _build_bass():
    nc = bacc.Bacc("TRN2", target_bir_lowering=False, debug=False,
                   num_devices=N_CORES)

    xt_d = nc.dram_tensor("xt", [NG, KP, WT, DPC + 2, H + 2], BF16,
                          kind="ExternalInput").ap()
    wt_d = nc.dram_tensor("wt", [NG, KP, 9, MP], BF16,
                          kind="ExternalInput").ap()
    pw_d = nc.dram_tensor("pwk", [C, F], BF16, kind="ExternalInput").ap()
    s1_d = nc.dram_tensor("s1", [MP, NG], F32, kind="ExternalInput").ap()
    b1_d = nc.dram_tensor("b1", [MP, NG], F32, kind="ExternalInput").ap()
    s2_d = nc.dram_tensor("s2", [F, 1], F32, kind="ExternalInput").ap()
    b2_d = nc.dram_tensor("b2", [F, 1], F32, kind="ExternalInput").ap()
    z_d = nc.dram_tensor("z", [F, NPOS], F32, kind="ExternalOutput").ap()

    with tile.TileContext(nc) as tc:
        with (
            tc.tile_pool(name="consts", bufs=1) as consts,
            tc.tile_pool(name="xt", bufs=NG) as xt_pool,
            tc.tile_pool(name="wt", bufs=NG) as wt_pool,
            tc.tile_pool(name="ybuf", bufs=3) as y_pool,
            tc.tile_pool(name="Ybig", bufs=1) as Y_pool,
            tc.tile_pool(name="zbuf", bufs=3) as z_pool,
            tc.tile_pool(name="psum", bufs=8, space="PSUM") as ps_pool,
        ):
            pw_sb = consts.tile([C, F], BF16)
            nc.sync.dma_start(pw_sb[:], pw_d[:])
            s1_sb = consts.tile([MP, NG], F32)
            nc.sync.dma_start(s1_sb[:], s1_d[:])
            b1_sb = consts.tile([MP, NG], F32)
            nc.sync.dma_start(b1_sb[:], b1_d[:])
            s2_sb = consts.tile([F, 1], F32)
            nc.sync.dma_start(s2_sb[:], s2_d[:])
            b2_sb = consts.tile([F, 1], F32)
            nc.sync.dma_start(b2_sb[:], b2_d[:])

            # Y: depthwise output in pure-channel layout.
            # free order (w_o, t, d, h): w_global = t*WO + w_o
            Y = Y_pool.tile([C, WO, WT, DPC, H], BF16)

            xg = []
            wg = []
            for g in range(NG):
                xg_t = xt_pool.tile([KP, WT, DPC + 2, H + 2], BF16, tag="xg")
                nc.sync.dma_start(xg_t[:], xt_d[g])
                wg_t = wt_pool.tile([KP, 9, MP], BF16, tag="wg")
                nc.sync.dma_start(wg_t[:], wt_d[g])
                xg.append(xg_t)
                wg.append(wg_t)

            for g in range(NG):
                yg = y_pool.tile([MP, WT, DPC, H], BF16, tag="yg")
                ps = [[ps_pool.tile([MP, DPC // NHALF, H], F32, tag="ps",
                                    name=f"ps_{t}_{nh}")
                       for nh in range(NHALF)] for t in range(WT)]
                for izy, (dz, dy) in enumerate(
                        (a, b) for a in range(3) for b in range(3)):
                    for t in range(WT):
                        for nh in range(NHALF):
                            d0 = nh * (DPC // NHALF)
                            rhs = xg[g][:, t, dz + d0: dz + d0 + DPC // NHALF,
                                        dy: dy + H]
                            nc.tensor.matmul(
                                ps[t][nh][:],
                                wg[g][:, izy, :],
                                rhs,
                                start=(izy == 0),
                                stop=(izy == 8),
                            )
                for t in range(WT):
                    for nh in range(NHALF):
                        d0 = nh * (DPC // NHALF)
                        nc.scalar.activation(
                            yg[:, t, d0: d0 + DPC // NHALF, :],
                            ps[t][nh][:],
                            mybir.ActivationFunctionType.Relu,
                            bias=b1_sb[:, g: g + 1],
                            scale=s1_sb[:, g: g + 1],
                        )
                # regroup (c,w)-partitions -> channel partitions.
                # src iter: (c, w_o, t, d, h) == dst free layout order
                nc.sync.dma_start(Y[g * CG:(g + 1) * CG], yg[:])

            # pointwise + BN2 + ReLU, chunked over positions
            n_chunks = NPOS // NCOL          # 96
            Yf = Y.rearrange("c w t d h -> c (w t d h)")
            zf = z_d.rearrange("f (j n) -> f j n", j=n_chunks, n=NCOL)
            for j0 in range(0, n_chunks, ZCHUNK):
                zt = z_pool.tile([F, ZCHUNK, NCOL], F32, tag="zt")
                for jj in range(ZCHUNK):
                    j = j0 + jj
                    pps = ps_pool.tile([F, NCOL], F32, tag="ps")
                    nc.tensor.matmul(
                        pps[:], pw_sb[:], Yf[:, j * NCOL:(j + 1) * NCOL],
                        start=True, stop=True)
                    nc.scalar.activation(
                        zt[:, jj, :], pps[:],
                        mybir.ActivationFunctionType.Relu,
                        bias=b2_sb[:, 0:1], scale=s2_sb[:, 0:1])
                nc.sync.dma_start(zf[:, j0: j0 + ZCHUNK, :], zt[:])

    nc.compile()
    return nc


def _prep_inputs(x, dw_kernel, dw_bias, bn1_gamma, bn1_beta, bn1_mean,
                 bn1_var, pw_kernel, pw_bias, bn2_gamma, bn2_beta, bn2_mean,
                 bn2_var):
    """Build per-core input maps (numpy only, off the device clock)."""
    x = np.asarray(x, np.float32)
    dw_kernel = np.asarray(dw_kernel, np.float32)
    dw_bias = np.asarray(dw_bias, np.float32)
    bn1_gamma = np.asarray(bn1_gamma, np.float32)
    bn1_beta = np.asarray(bn1_beta, np.float32)
    bn1_mean = np.asarray(bn1_mean, np.float32)
    bn1_var = np.asarray(bn1_var, np.float32)
    pw_kernel = np.asarray(pw_kernel, np.float32)
    pw_bias = np.asarray(pw_bias, np.float32)
    bn2_gamma = np.asarray(bn2_gamma, np.float32)
    bn2_beta = np.asarray(bn2_beta, np.float32)
    bn2_mean = np.asarray(bn2_mean, np.float32)
    bn2_var = np.asarray(bn2_var, np.float32)
    a1 = bn1_gamma / np.sqrt(bn1_var + EPS)
    c1 = a1 * (dw_bias - bn1_mean) + bn1_beta
    a2 = bn2_gamma / np.sqrt(bn2_var + EPS)
    c2 = a2 * (pw_bias - bn2_mean) + bn2_beta

    # depthwise Toeplitz weights: [NG, KP, 9, MP]
    dw = dw_kernel[:, :, :, 0, :]                       # [3,3,3,C]
    wt = np.zeros((NG, KP, 9, MP), np.float32)
    for ci in range(CG):
        for wo in range(WO):
            for dx in range(3):
                # wt[g, ci*WI + wo+dx, (dz*3+dy), ci*WO + wo] = dw[dz,dy,dx,c]
                wt[:, ci * WI + wo + dx, :, ci * WO + wo] = (
                    dw[:, :, dx, :].reshape(9, NG, CG)[:, :, ci].T)
    wt = wt.astype(ml_dtypes.bfloat16)

    # scale/bias vectors in (c-major, w) partition order: m = ci*WO + wo
    s1 = np.zeros((MP, NG), np.float32)
    b1 = np.zeros((MP, NG), np.float32)
    for g in range(NG):
        for ci in range(CG):
            s1[ci * WO:(ci + 1) * WO, g] = a1[g * CG + ci]
            b1[ci * WO:(ci + 1) * WO, g] = c1[g * CG + ci]

    pwk = pw_kernel.astype(ml_dtypes.bfloat16)
    s2 = a2.reshape(F, 1).astype(np.float32)
    b2 = c2.reshape(F, 1).astype(np.float32)

    # x padded once globally: [B, D+2, H+2, W+2, C]
    xp = np.zeros((B, D + 2, H + 2, W + 2, C), np.float32)
    xp[:, 1:-1, 1:-1, 1:-1, :] = x
    xp = xp.astype(ml_dtypes.bfloat16)

    in_maps = []
    for core in range(N_CORES):
        b = (core * DPC) // D
        d0 = (core * DPC) % D
        sl = xp[b, d0: d0 + DPC + 2]                    # [14, 50, 50, C]
        # xt[g, ci*WI+wi, t, d, h] = sl[d, h, 12t+wi, 8g+ci]
        xt = np.ascontiguousarray(sl.transpose(3, 2, 0, 1))  # [C, w50, d, h]
        # build overlapping w-tiles: index w = t*WO + wi
        idx = (np.arange(WT)[:, None] * WO + np.arange(WI)[None, :]).ravel()
        xt = xt[:, idx]                                 # [C, WT*WI, d, h]
        xt = xt.reshape(NG, CG, WT, WI, DPC + 2, H + 2) \
               .transpose(0, 1, 3, 2, 4, 5) \
               .reshape(NG, KP, WT, DPC + 2, H + 2)
        in_maps.append({
            "xt": np.ascontiguousarray(xt),
            "wt": wt, "pwk": pwk, "s1": s1, "b1": b1, "s2": s2, "b2": b2,
        })
    return in_maps


def _gather_output(results):
    z = np.empty((B, D, H, W, F), np.float32)
    for core in range(N_CORES):
        b = (core * DPC) // D
        d0 = (core * DPC) % D
        zc = results[core]["z"]                         # [F, NPOS]
        # free order was (w_o, t, d, h); w_global = t*WO + w_o
        zc = zc.reshape(F, WO, WT, DPC, H).transpose(3, 4, 2, 1, 0)
        z[b, d0: d0 + DPC] = zc.reshape(DPC, H, W, F)
    return z


def kernel(**inputs):
    global _COMPILED
    if _COMPILED is None:
        _COMPILED = _build_bass()
    in_maps = _prep_inputs(**inputs)
    res = run_bass_kernel_spmd(_COMPILED, in_maps,
                               core_ids=list(range(N_CORES)))
    return _gather_output(res.results)


if __name__ == "__main__":
    pass
